# revision 14
# baseline (speedup 1.0000x reference)
"""DCNv2 (deformable conv) Trainium2 kernel.

Strategy (data-parallel over batch, one sample per NeuronCore):
  Host: pad x to 128x128, build a channels-last "quad image" where row
  (y,x) holds the 2x2 bilinear corner patch for all 128 channels
  (fp16, 1KB rows). Compute int16 gather indices and the 4 bilinear
  corner weights per (tap k, pixel) from `offset`.
  Device, per 2304-pixel stripe x 9 taps:
    dma_gather (SWDGE)   -> G [128 pix, 18 blk, 4*128] fp16
      - gather calls round-robin over 4 SWDGE queues: queue q's
        descriptor generation runs on Q7 core pair (2q, 2q+1), so 4
        queues parallelize desc-gen ~4x (the baseline bottleneck).
    4x DVE mul (in-place, weights broadcast along channels via
                dup-pair stride-0 APs)
    3x DVE add           -> S_T [pix, c] per tap
    PE transpose per 128-block, batched 8-to-a-PSUM-bank; one ACT
      copy per bank -> S [c, pix] per tap
    9-tap GEMM (k-outer) accumulating into 6 persistent PSUM fp32
      chunks per stripe -> out [o, pix]; one output DMA per stripe
"""

import numpy as np

import concourse.mybir as mybir
import concourse.tile as tile
from concourse import bacc, bass_utils, library_config
from concourse.masks import make_identity

P = 128
B, C, H, W, KK = 8, 128, 96, 96, 3
HW = H * W                  # 9216
NK = KK * KK                # 9
PAD = 16
HP = WP = 128
NROW = HP * WP              # 16384 quad-image rows
ELEM = 4 * C                # 512 fp16 elems per quad row (1KB)
NSTR = 3                    # pixel stripes
SPIX = HW // NSTR           # 3072 pixels per stripe
NBLK = SPIX // P            # 24
NCH = 6                     # GEMM n-chunks per stripe
CHW = SPIX // NCH           # 512
ICOL = SPIX // 16           # 192 wrapped-idx columns per (k, stripe)
NQ = 4                      # SWDGE queues (desc-gen core pairs)

F16, F32, I16 = mybir.dt.float16, mybir.dt.float32, mybir.dt.int16

TRACE = False               # set by test harness to capture a profile
LAST_RESULTS = None

_CACHE = {}


def _build(reps=1, variant="a"):
    key = ("nc", reps, variant)
    if key in _CACHE:
        return _CACHE[key]
    nc = bacc.Bacc("TRN2", target_bir_lowering=False, debug=False,
                   enable_asserts=False, num_swdge_queues=NQ)
    xq_d = nc.dram_tensor("xq", [NROW, ELEM], F16, kind="ExternalInput")
    idx_d = nc.dram_tensor("idx", [P, NK * NSTR * ICOL], I16,
                           kind="ExternalInput")
    wts_d = nc.dram_tensor("wts", [P, NK * NSTR * NBLK * 8], F16,
                           kind="ExternalInput")
    w2_d = nc.dram_tensor("w2", [P, NK * P], F16, kind="ExternalInput")
    out_d = nc.dram_tensor("out", [P, HW], F32, kind="ExternalOutput")

    with (
        tile.TileContext(nc) as tc,
        tc.tile_pool(name="const", bufs=1) as const_p,
        tc.tile_pool(name="g", bufs=3) as g_p,
        tc.tile_pool(name="gw", bufs=2) as gw_p,
        tc.tile_pool(name="scur", bufs=2) as s_p,
        tc.tile_pool(name="ob", bufs=1) as out_p,
        tc.tile_pool(name="tp", bufs=2, space="PSUM") as tp_p,
        tc.tile_pool(name="mm", bufs=NCH, space="PSUM") as mm_p,
    ):
        nc.gpsimd.load_library(library_config.mlp)
        ident = const_p.tile([P, P], F16)
        make_identity(nc, ident[:])
        idx_sb = const_p.tile([P, NK, NSTR, ICOL], I16)
        nc.sync.dma_start(idx_sb[:], idx_d[:])
        wts_sb = const_p.tile([P, NK, NSTR, NBLK, 4, 2], F16)
        nc.sync.dma_start(wts_sb[:], wts_d[:])
        w2_sb = const_p.tile([P, NK, P], F16)
        nc.sync.dma_start(w2_sb[:], w2_d[:])

        call_i = 0
        for rep in range(reps):
          for s in range(NSTR):
              mm = [mm_p.tile([P, CHW], F32, name="mmacc")
                    for _ in range(NCH)]
              for k in range(NK):
                  g_t = g_p.tile([P, NBLK, ELEM], F16)
                  # chunked gathers: 1024 idxs/call = 64 descs/engine,
                  # exactly at the packet ceiling
                  gch = 3
                  cpix = SPIX // gch           # 1024
                  cblk = NBLK // gch           # 8
                  ccol = ICOL // gch           # 64
                  for c in range(gch):
                      nc.gpsimd.dma_gather(
                          g_t[:, c * cblk:(c + 1) * cblk, :], xq_d[:],
                          idx_sb[:, k, s, c * ccol:(c + 1) * ccol],
                          cpix, cpix, ELEM, queue_num=call_i % NQ)
                      call_i += 1
                  # weighted corners -> corner-major gw [P, 4, SPIX] so the
                  # corner-sum adds below are flat-contiguous (2x DVE mode).
                  # Muls are per gather-chunk so DVE starts as soon as each
                  # 1024-pixel gather lands (slice-precise tile deps).
                  gw_t = gw_p.tile([P, 4, SPIX], F16)
                  for c in range(gch):
                      blks = slice(c * cblk, (c + 1) * cblk)
                      for c_ in range(4):
                          v = g_t[:, blks, c_ * P:(c_ + 1) * P].rearrange(
                              "p b (r d) -> p b r d", d=2)
                          w_ap = wts_sb[:, k, s, blks, c_:c_ + 1, :
                                        ].to_broadcast([P, cblk, P // 2, 2])
                          o = gw_t[:, c_, c * cpix:(c + 1) * cpix].rearrange(
                              "p (b r d) -> p b r d", b=cblk, d=2)
                          nc.vector.tensor_tensor(out=o, in0=v, in1=w_ap,
                                                  op=mybir.AluOpType.mult)
                  nc.vector.tensor_add(out=gw_t[:, 0], in0=gw_t[:, 0],
                                       in1=gw_t[:, 1])
                  nc.vector.tensor_add(out=gw_t[:, 2], in0=gw_t[:, 2],
                                       in1=gw_t[:, 3])
                  nc.vector.tensor_add(out=gw_t[:, 0], in0=gw_t[:, 0],
                                       in1=gw_t[:, 2])
                  st_t = gw_t[:, 0].rearrange("p (b q) -> p b q", b=NBLK)
                  # transpose to [c, pix]: batch 8 blocks per PSUM bank,
                  # one ACT copy per bank
                  s_sb = s_p.tile([P, SPIX], F16)
                  for g0 in range(0, NBLK, 8):
                      nb = min(8, NBLK - g0)
                      tp_t = tp_p.tile([P, 8 * P], F16)
                      for b in range(nb):
                          nc.tensor.transpose(
                              out=tp_t[:, b * P:(b + 1) * P],
                              in_=st_t[:, g0 + b], identity=ident[:])
                      nc.scalar.copy(out=s_sb[:, g0 * P:(g0 + nb) * P],
                                     in_=tp_t[:, 0:nb * P])
                  # 9-tap GEMM, k-outer: accumulate all 6 chunks per tap
                  for n in range(NCH):
                      nc.tensor.matmul(out=mm[n][:], lhsT=w2_sb[:, k],
                                       rhs=s_sb[:, n * CHW:(n + 1) * CHW],
                                       start=(k == 0), stop=(k == NK - 1),
                                       skip_group_check=True)
              o_sb = out_p.tile([P, SPIX], F32)
              for n in range(NCH):
                  nc.scalar.copy(out=o_sb[:, n * CHW:(n + 1) * CHW],
                                 in_=mm[n][:])
              nc.sync.dma_start(
                  out_d[:, s * SPIX:(s + 1) * SPIX], o_sb[:])
    nc.compile()
    _CACHE[key] = nc
    return nc


def _host_prep(x, offset, weight):
    x = np.asarray(x, dtype=np.float32)
    offset = np.asarray(offset, dtype=np.float32)
    weight = np.asarray(weight, dtype=np.float32)

    # quad image [B, NROW, 4*C] fp16, zero padded
    xt = np.zeros((B, HP + 1, WP + 1, C), np.float16)
    xt[:, PAD:PAD + H, PAD:PAD + W, :] = np.transpose(
        x, (0, 2, 3, 1)).astype(np.float16)
    quad = np.stack([xt[:, :HP, :WP], xt[:, :HP, 1:],
                     xt[:, 1:, :WP], xt[:, 1:, 1:]], axis=3)
    xq = np.ascontiguousarray(quad.reshape(B, NROW, ELEM))

    # sampling positions (float32, matching the reference exactly)
    off = offset.reshape(B, NK, 2, H, W)
    oy = np.arange(H, dtype=np.float32).reshape(1, 1, H, 1)
    ox = np.arange(W, dtype=np.float32).reshape(1, 1, 1, W)
    kh = (np.arange(NK) // KK).astype(np.float32).reshape(1, NK, 1, 1)
    kw = (np.arange(NK) % KK).astype(np.float32).reshape(1, NK, 1, 1)
    py = oy - 1.0 + kh + off[:, :, 0]
    px = ox - 1.0 + kw + off[:, :, 1]
    y0 = np.floor(py)
    x0 = np.floor(px)
    dy = py - y0
    dx = px - x0
    ry = np.clip(y0.astype(np.int32) + PAD, 0, HP - 2)
    rx = np.clip(x0.astype(np.int32) + PAD, 0, WP - 2)
    idx = (ry * WP + rx).astype(np.int16)                    # [B,NK,H,W]

    # wrapped gather indices: [B, 128, NK*NSTR*ICOL]
    idxf = idx.reshape(B, NK, NSTR, ICOL, 16)
    idxw = idxf.transpose(0, 1, 2, 4, 3)                     # [B,NK,NSTR,16,ICOL]
    idxw = np.broadcast_to(idxw[:, :, :, None],
                           (B, NK, NSTR, 8, 16, ICOL))
    idx_host = np.ascontiguousarray(
        idxw.transpose(0, 3, 4, 1, 2, 5).reshape(B, P, NK * NSTR * ICOL))

    # corner weights [B, 128, NK*NSTR*NBLK*4*2] fp16 (dup pairs)
    w4 = np.stack([(1 - dy) * (1 - dx), (1 - dy) * dx,
                   dy * (1 - dx), dy * dx], axis=-1).astype(np.float16)
    w5 = w4.reshape(B, NK, NSTR, NBLK, P, 4)
    w_host = w5.transpose(0, 4, 1, 2, 3, 5)                  # [B,P,NK,NSTR,NBLK,4]
    w_host = np.ascontiguousarray(
        np.repeat(w_host[..., None], 2, axis=-1).reshape(
            B, P, NK * NSTR * NBLK * 8))

    # GEMM weights: lhsT per tap = W_k^T [c, o]
    w2h = weight.reshape(C, C, NK).transpose(2, 1, 0).astype(np.float16)
    w2_host = np.ascontiguousarray(w2h.transpose(1, 0, 2).reshape(P, NK * P))
    return xq, idx_host, w_host, w2_host


def kernel(x, offset, weight):
    global LAST_RESULTS
    nc = _build()
    xq, idx_host, w_host, w2_host = _host_prep(x, offset, weight)
    in_maps = [
        {"xq": xq[b], "idx": idx_host[b], "wts": w_host[b], "w2": w2_host}
        for b in range(B)
    ]
    res = bass_utils.run_bass_kernel_spmd(
        nc, in_maps, core_ids=list(range(B)), trace=TRACE)
    LAST_RESULTS = res
    out = np.stack([res.results[b]["out"] for b in range(B)])
    return out.reshape(B, C, H, W).astype(np.float32)


# revision 15
# speedup vs baseline: 1.2899x; 1.2899x over previous
"""DCNv2 (deformable conv) Trainium2 kernel.

Strategy (data-parallel over batch, one sample per NeuronCore):
  Host: pad x to 128x128, build a channels-last "quad image" where row
  (y,x) holds the 2x2 bilinear corner patch for all 128 channels
  (fp16, 1KB rows). Compute int16 gather indices and the 4 bilinear
  corner weights per (tap k, pixel) from `offset`.
  Device, per 2304-pixel stripe x 9 taps:
    dma_gather (SWDGE)   -> G [128 pix, 18 blk, 4*128] fp16
      - gather calls round-robin over 4 SWDGE queues: queue q's
        descriptor generation runs on Q7 core pair (2q, 2q+1), so 4
        queues parallelize desc-gen ~4x (the baseline bottleneck).
    4x DVE mul (in-place, weights broadcast along channels via
                dup-pair stride-0 APs)
    3x DVE add           -> S_T [pix, c] per tap
    PE transpose per 128-block, batched 8-to-a-PSUM-bank; one ACT
      copy per bank -> S [c, pix] per tap
    9-tap GEMM (k-outer) accumulating into 6 persistent PSUM fp32
      chunks per stripe -> out [o, pix]; one output DMA per stripe
"""

import numpy as np

import concourse.mybir as mybir
import concourse.tile as tile
from concourse import bacc, bass_utils, library_config
from concourse.masks import make_identity

P = 128
B, C, H, W, KK = 8, 128, 96, 96, 3
HW = H * W                  # 9216
NK = KK * KK                # 9
PAD = 16
HP = WP = 128
NROW = HP * WP              # 16384 quad-image rows
ELEM = 4 * C                # 512 fp16 elems per quad row (1KB)
NSTR = 3                    # pixel stripes
SPIX = HW // NSTR           # 3072 pixels per stripe
NBLK = SPIX // P            # 24
NCH = 6                     # GEMM n-chunks per stripe
CHW = SPIX // NCH           # 512
ICOL = SPIX // 16           # 192 wrapped-idx columns per (k, stripe)
NQ = 4                      # SWDGE queues (desc-gen core pairs)

F16, F32, I16 = mybir.dt.float16, mybir.dt.float32, mybir.dt.int16

TRACE = False               # set by test harness to capture a profile
LAST_RESULTS = None

_CACHE = {}


def _build(reps=1, variant="a"):
    key = ("nc", reps, variant)
    if key in _CACHE:
        return _CACHE[key]
    nc = bacc.Bacc("TRN2", target_bir_lowering=False, debug=False,
                   enable_asserts=False, num_swdge_queues=NQ)
    xq_d = nc.dram_tensor("xq", [NROW, ELEM], F16, kind="ExternalInput")
    idx_d = nc.dram_tensor("idx", [P, NK * NSTR * ICOL], I16,
                           kind="ExternalInput")
    wts_d = nc.dram_tensor("wts", [P, NK * NSTR * NBLK * 8], F16,
                           kind="ExternalInput")
    w2_d = nc.dram_tensor("w2", [P, NK * P], F16, kind="ExternalInput")
    out_d = nc.dram_tensor("out", [P, HW], F32, kind="ExternalOutput")

    with (
        tile.TileContext(nc) as tc,
        tc.tile_pool(name="const", bufs=1) as const_p,
        tc.tile_pool(name="g", bufs=3) as g_p,
        tc.tile_pool(name="gw", bufs=2) as gw_p,
        tc.tile_pool(name="scur", bufs=2) as s_p,
        tc.tile_pool(name="ob", bufs=1) as out_p,
        tc.tile_pool(name="tp", bufs=2, space="PSUM") as tp_p,
        tc.tile_pool(name="mm", bufs=NCH, space="PSUM") as mm_p,
    ):
        nc.gpsimd.load_library(library_config.mlp)
        ident = const_p.tile([P, P], F16)
        make_identity(nc, ident[:])
        idx_sb = const_p.tile([P, NK, NSTR, ICOL], I16)
        nc.sync.dma_start(idx_sb[:], idx_d[:])
        wts_sb = const_p.tile([P, NK, NSTR, NBLK, 4, 2], F16)
        nc.sync.dma_start(wts_sb[:], wts_d[:])
        w2_sb = const_p.tile([P, NK, P], F16)
        nc.sync.dma_start(w2_sb[:], w2_d[:])

        call_i = 0
        for rep in range(reps):
          for s in range(NSTR):
              mm = [mm_p.tile([P, CHW], F32, name="mmacc")
                    for _ in range(NCH)]
              for k in range(NK):
                  g_t = g_p.tile([P, NBLK, ELEM], F16)
                  # chunked gathers: 1024 idxs/call = 64 descs/engine,
                  # exactly at the packet ceiling
                  gch = 3
                  cpix = SPIX // gch           # 1024
                  cblk = NBLK // gch           # 8
                  ccol = ICOL // gch           # 64
                  for c in range(gch):
                      nc.gpsimd.dma_gather(
                          g_t[:, c * cblk:(c + 1) * cblk, :], xq_d[:],
                          idx_sb[:, k, s, c * ccol:(c + 1) * ccol],
                          cpix, cpix, ELEM, queue_num=call_i % NQ)
                      call_i += 1
                  # weighted corners -> corner-major gw [P, 4, SPIX] so the
                  # corner-sum adds below are flat-contiguous (2x DVE mode)
                  gw_t = gw_p.tile([P, 4, SPIX], F16)
                  for c_ in range(4):
                      v = g_t[:, :, c_ * P:(c_ + 1) * P].rearrange(
                          "p b (r d) -> p b r d", d=2)
                      w_ap = wts_sb[:, k, s, :, c_:c_ + 1, :].to_broadcast(
                          [P, NBLK, P // 2, 2])
                      o = gw_t[:, c_].rearrange("p (b r d) -> p b r d",
                                                b=NBLK, d=2)
                      nc.vector.tensor_tensor(out=o, in0=v, in1=w_ap,
                                              op=mybir.AluOpType.mult)
                  nc.vector.tensor_add(out=gw_t[:, 0], in0=gw_t[:, 0],
                                       in1=gw_t[:, 1])
                  nc.vector.tensor_add(out=gw_t[:, 2], in0=gw_t[:, 2],
                                       in1=gw_t[:, 3])
                  nc.vector.tensor_add(out=gw_t[:, 0], in0=gw_t[:, 0],
                                       in1=gw_t[:, 2])
                  st_t = gw_t[:, 0].rearrange("p (b q) -> p b q", b=NBLK)
                  # transpose to [c, pix]: batch 8 blocks per PSUM bank,
                  # one ACT copy per bank
                  s_sb = s_p.tile([P, SPIX], F16)
                  for g0 in range(0, NBLK, 8):
                      nb = min(8, NBLK - g0)
                      tp_t = tp_p.tile([P, 8 * P], F16)
                      for b in range(nb):
                          nc.tensor.transpose(
                              out=tp_t[:, b * P:(b + 1) * P],
                              in_=st_t[:, g0 + b], identity=ident[:])
                      nc.scalar.copy(out=s_sb[:, g0 * P:(g0 + nb) * P],
                                     in_=tp_t[:, 0:nb * P])
                  # 9-tap GEMM, k-outer: accumulate all 6 chunks per tap
                  for n in range(NCH):
                      nc.tensor.matmul(out=mm[n][:], lhsT=w2_sb[:, k],
                                       rhs=s_sb[:, n * CHW:(n + 1) * CHW],
                                       start=(k == 0), stop=(k == NK - 1),
                                       skip_group_check=True)
              o_sb = out_p.tile([P, SPIX], F32)
              for n in range(NCH):
                  nc.scalar.copy(out=o_sb[:, n * CHW:(n + 1) * CHW],
                                 in_=mm[n][:])
              nc.sync.dma_start(
                  out_d[:, s * SPIX:(s + 1) * SPIX], o_sb[:])
    nc.compile()
    _CACHE[key] = nc
    return nc


def _host_prep(x, offset, weight):
    x = np.asarray(x, dtype=np.float32)
    offset = np.asarray(offset, dtype=np.float32)
    weight = np.asarray(weight, dtype=np.float32)

    # quad image [B, NROW, 4*C] fp16, zero padded
    xt = np.zeros((B, HP + 1, WP + 1, C), np.float16)
    xt[:, PAD:PAD + H, PAD:PAD + W, :] = np.transpose(
        x, (0, 2, 3, 1)).astype(np.float16)
    quad = np.stack([xt[:, :HP, :WP], xt[:, :HP, 1:],
                     xt[:, 1:, :WP], xt[:, 1:, 1:]], axis=3)
    xq = np.ascontiguousarray(quad.reshape(B, NROW, ELEM))

    # sampling positions (float32, matching the reference exactly)
    off = offset.reshape(B, NK, 2, H, W)
    oy = np.arange(H, dtype=np.float32).reshape(1, 1, H, 1)
    ox = np.arange(W, dtype=np.float32).reshape(1, 1, 1, W)
    kh = (np.arange(NK) // KK).astype(np.float32).reshape(1, NK, 1, 1)
    kw = (np.arange(NK) % KK).astype(np.float32).reshape(1, NK, 1, 1)
    py = oy - 1.0 + kh + off[:, :, 0]
    px = ox - 1.0 + kw + off[:, :, 1]
    y0 = np.floor(py)
    x0 = np.floor(px)
    dy = py - y0
    dx = px - x0
    ry = np.clip(y0.astype(np.int32) + PAD, 0, HP - 2)
    rx = np.clip(x0.astype(np.int32) + PAD, 0, WP - 2)
    idx = (ry * WP + rx).astype(np.int16)                    # [B,NK,H,W]

    # wrapped gather indices: [B, 128, NK*NSTR*ICOL]
    idxf = idx.reshape(B, NK, NSTR, ICOL, 16)
    idxw = idxf.transpose(0, 1, 2, 4, 3)                     # [B,NK,NSTR,16,ICOL]
    idxw = np.broadcast_to(idxw[:, :, :, None],
                           (B, NK, NSTR, 8, 16, ICOL))
    idx_host = np.ascontiguousarray(
        idxw.transpose(0, 3, 4, 1, 2, 5).reshape(B, P, NK * NSTR * ICOL))

    # corner weights [B, 128, NK*NSTR*NBLK*4*2] fp16 (dup pairs)
    w4 = np.stack([(1 - dy) * (1 - dx), (1 - dy) * dx,
                   dy * (1 - dx), dy * dx], axis=-1).astype(np.float16)
    w5 = w4.reshape(B, NK, NSTR, NBLK, P, 4)
    w_host = w5.transpose(0, 4, 1, 2, 3, 5)                  # [B,P,NK,NSTR,NBLK,4]
    w_host = np.ascontiguousarray(
        np.repeat(w_host[..., None], 2, axis=-1).reshape(
            B, P, NK * NSTR * NBLK * 8))

    # GEMM weights: lhsT per tap = W_k^T [c, o]
    w2h = weight.reshape(C, C, NK).transpose(2, 1, 0).astype(np.float16)
    w2_host = np.ascontiguousarray(w2h.transpose(1, 0, 2).reshape(P, NK * P))
    return xq, idx_host, w_host, w2_host


def kernel(x, offset, weight):
    global LAST_RESULTS
    nc = _build()
    xq, idx_host, w_host, w2_host = _host_prep(x, offset, weight)
    in_maps = [
        {"xq": xq[b], "idx": idx_host[b], "wts": w_host[b], "w2": w2_host}
        for b in range(B)
    ]
    res = bass_utils.run_bass_kernel_spmd(
        nc, in_maps, core_ids=list(range(B)), trace=TRACE)
    LAST_RESULTS = res
    out = np.stack([res.results[b]["out"] for b in range(B)])
    return out.reshape(B, C, H, W).astype(np.float32)
